# revision 3
# baseline (speedup 1.0000x reference)
"""Trainium2 Bass kernel for DisentangledMultiHeadAttention.

Problem (full size): x (8, 2048, 4096), Wq/Wk/Wv/Wo (4096, 4096),
position_bias (2, 2048). Returns (y, atten) like the reference:
  q,k,v = x@W{q,k,v}.T split into 2 heads of 2048
  scores = q@k^T / sqrt(D) + position_bias (broadcast over key axis)
  atten = softmax(scores); y = (atten@v) @ Wo.T

Sharding: data-parallel over batch B=8 across the 8 NeuronCores (one
batch element per core, full weights on every core).

All matmuls run on the PE in float32r (tf32-like, ~1e-4 relative error,
full bf16-rate at free-dim >= 256). Host pre-transposes x and the
weights so every matmul operand has the contraction dim on partitions.

Per-core dataflow (T=2048 tokens, C=4096, H=2 heads, D=T=2048):
  A) projections (x stationary in SBUF per t-half):
       qT[d,t] = scale * (x@Wq.T).T, kT[d,t], v[s,d]  -> DRAM (f32r)
  B) per head h:
     B1) per 128-token tile: scoresT... scores[t,s] psum = qT.T@kT tiles,
         += bias (DVE), Exp via ACT (accum_out row sums), normalize
         (per-partition reciprocal), DMA atten out, PE-transpose the
         normalized atten into attT[s,t] -> DRAM
     B2) attnoutT[d,t] = v.T-tiles @ attT -> DRAM (attn_flatT layout)
  C) y[t,e] = attn_flatT.T @ WoT, streaming Wo e-chunks.
"""

import sys
import os

sys.path.insert(0, "/opt/trn_rl_repo")

import numpy as np
from contextlib import ExitStack

import concourse.bass as bass
import concourse.mybir as mybir
import concourse.tile as tile
from concourse import bacc
from concourse.bass_utils import run_bass_kernel_spmd
from concourse.masks import make_identity

P = 128
F32 = mybir.dt.float32
F32R = mybir.dt.float32r

# Full-size problem constants
FULL_B, FULL_T, FULL_C, FULL_H = 8, 2048, 4096, 2


def build_program(T=FULL_T, C=FULL_C, H=FULL_H, num_devices=8):
    """Build the per-core SPMD program. Requires T == C // H (bias
    broadcast), T % 256 == 0, C % 256 == 0."""
    D = C // H
    assert D == T
    CO = C // P           # contraction tiles over the embedding dim
    TO = T // P           # token tiles
    DO = D // P           # head-dim tiles
    TH = T // 2           # token half (x / attn_flat residency unit)
    NQ = min(512, TH)     # projection output t-chunk
    SC = min(512, T)      # scores s-chunk / B2 t-chunk
    VN = 256              # v d-chunk (>=256 keeps f32r at full rate)
    EN = 256              # output-proj e-chunk
    scale = 1.0 / float(np.sqrt(D))

    nc = bacc.Bacc("TRN2", num_devices=num_devices)

    xT = nc.dram_tensor("xT", [P, CO, T], F32R, kind="ExternalInput")
    wqT = nc.dram_tensor("wqT", [P, CO, C], F32R, kind="ExternalInput")
    wkT = nc.dram_tensor("wkT", [P, CO, C], F32R, kind="ExternalInput")
    wvT = nc.dram_tensor("wvT", [P, CO, C], F32R, kind="ExternalInput")
    woT = nc.dram_tensor("woT", [P, CO, C], F32R, kind="ExternalInput")
    bias_rep = nc.dram_tensor("bias_rep", [H, P, T], F32, kind="ExternalInput")

    atten_o = nc.dram_tensor("atten_o", [H, T, T], F32, kind="ExternalOutput")
    y_o = nc.dram_tensor("y_o", [T, C], F32, kind="ExternalOutput")

    qT_d = nc.dram_tensor("qT_d", [P, CO, T], F32R)
    kT_d = nc.dram_tensor("kT_d", [P, CO, T], F32R)
    v_d = nc.dram_tensor("v_d", [P, TO, C], F32R)
    attT_d = nc.dram_tensor("attT_d", [P, TO, T], F32R)
    aoT_d = nc.dram_tensor("aoT_d", [P, CO, T], F32R)

    with tile.TileContext(nc) as tc, ExitStack() as ctx:
        cpool = ctx.enter_context(tc.tile_pool(name="const", bufs=1))
        ident_f = cpool.tile([P, P], F32)
        make_identity(nc, ident_f[:])
        ident = cpool.tile([P, P], F32R)
        nc.vector.tensor_copy(ident[:], ident_f[:])

        # ---------------- Stage A: projections ----------------
        for hf in range(2):
            t0 = hf * TH
            with tc.tile_pool(name="A_x", bufs=1) as px:
                xh = px.tile([P, CO, TH], F32R)
                nc.sync.dma_start(xh[:], xT[:, :, t0:t0 + TH])

                with (
                    tc.tile_pool(name="A_wqk", bufs=2) as pw,
                    tc.tile_pool(name="A_st", bufs=4) as pst,
                    tc.tile_pool(name="A_ps", bufs=4, space="PSUM") as pps,
                ):
                    for w_in, spill, do_scale in (
                        (wqT, qT_d, True),
                        (wkT, kT_d, False),
                    ):
                        for do in range(CO):
                            wsl = pw.tile([P, CO, P], F32R, tag="wqk")
                            nc.sync.dma_start(wsl[:], w_in[:, :, do * P:(do + 1) * P])
                            for tc_i in range(TH // NQ):
                                ps = pps.tile([P, NQ], F32, tag="psA")
                                for co in range(CO):
                                    nc.tensor.matmul(
                                        ps[:], wsl[:, co], xh[:, co, tc_i * NQ:(tc_i + 1) * NQ],
                                        start=(co == 0), stop=(co == CO - 1))
                                st = pst.tile([P, NQ], F32R, tag="stA")
                                if do_scale:
                                    nc.vector.tensor_scalar_mul(st[:], ps[:], scale)
                                else:
                                    nc.vector.tensor_copy(st[:], ps[:])
                                nc.sync.dma_start(
                                    spill[:, do, t0 + tc_i * NQ: t0 + (tc_i + 1) * NQ], st[:])

                with (
                    tc.tile_pool(name="A_wv", bufs=2) as pwv,
                    tc.tile_pool(name="A_stv", bufs=4) as pstv,
                    tc.tile_pool(name="A_psv", bufs=4, space="PSUM") as ppsv,
                ):
                    for dch in range(C // VN):
                        wsl = pwv.tile([P, CO, VN], F32R, tag="wv")
                        nc.sync.dma_start(wsl[:], wvT[:, :, dch * VN:(dch + 1) * VN])
                        for so in range(TH // P):
                            ps = ppsv.tile([P, VN], F32, tag="psV")
                            for co in range(CO):
                                nc.tensor.matmul(
                                    ps[:], xh[:, co, so * P:(so + 1) * P], wsl[:, co],
                                    start=(co == 0), stop=(co == CO - 1))
                            st = pstv.tile([P, VN], F32R, tag="stV")
                            nc.vector.tensor_copy(st[:], ps[:])
                            nc.sync.dma_start(
                                v_d[:, hf * (TH // P) + so, dch * VN:(dch + 1) * VN], st[:])

        # ---------------- Stage B: attention per head ----------------
        for h in range(H):
            # B1: scores -> softmax -> atten out + attT spill
            with (
                tc.tile_pool(name="B_k", bufs=1) as pk,
                tc.tile_pool(name="B_bias", bufs=1) as pb,
                tc.tile_pool(name="B_q", bufs=2) as pq,
                tc.tile_pool(name="B_e", bufs=2) as pe,
                tc.tile_pool(name="B_att", bufs=2) as pat,
                tc.tile_pool(name="B_r", bufs=4) as pr,
                tc.tile_pool(name="B_tr", bufs=4) as ptr,
                tc.tile_pool(name="B_ps", bufs=4, space="PSUM") as pps,
                tc.tile_pool(name="B_pst", bufs=2, space="PSUM") as ppst,
            ):
                kt = pk.tile([P, DO, T], F32R)
                nc.sync.dma_start(kt[:], kT_d[:, h * DO:(h + 1) * DO, :])
                bias_sb = pb.tile([P, T], F32)
                nc.sync.dma_start(bias_sb[:], bias_rep[h])

                for tt in range(TO):
                    qtile = pq.tile([P, DO, P], F32R, tag="q")
                    nc.sync.dma_start(
                        qtile[:], qT_d[:, h * DO:(h + 1) * DO, tt * P:(tt + 1) * P])
                    e_sb = pe.tile([P, T], F32, tag="E")
                    rs4 = pr.tile([P, T // SC], F32, tag="rs4")
                    for sch in range(T // SC):
                        ps = pps.tile([P, SC], F32, tag="psB")
                        for do in range(DO):
                            nc.tensor.matmul(
                                ps[:], qtile[:, do], kt[:, do, sch * SC:(sch + 1) * SC],
                                start=(do == 0), stop=(do == DO - 1))
                        nc.vector.tensor_add(
                            out=ps[:], in0=ps[:], in1=bias_sb[:, sch * SC:(sch + 1) * SC])
                        nc.scalar.activation(
                            e_sb[:, sch * SC:(sch + 1) * SC], ps[:],
                            mybir.ActivationFunctionType.Exp,
                            accum_out=rs4[:, sch:sch + 1])
                    rs = pr.tile([P, 1], F32, tag="rs")
                    nc.vector.reduce_sum(rs[:], rs4[:], axis=mybir.AxisListType.X)
                    rcp = pr.tile([P, 1], F32, tag="rcp")
                    nc.vector.reciprocal(rcp[:], rs[:])
                    att = pat.tile([P, T], F32R, tag="att")
                    nc.vector.tensor_scalar_mul(att[:], e_sb[:], rcp[:])
                    nc.sync.dma_start(
                        atten_o[h, tt * P:(tt + 1) * P, :], att.bitcast(F32))
                    for so in range(TO):
                        pst = ppst.tile([P, P], F32R, tag="ptr")
                        nc.tensor.transpose(pst[:], att[:, so * P:(so + 1) * P], ident[:])
                        trs = ptr.tile([P, P], F32R, tag="trs")
                        nc.vector.tensor_copy(trs[:], pst[:])
                        nc.sync.dma_start(attT_d[:, so, tt * P:(tt + 1) * P], trs[:])

            # B2: attnoutT = v.T-tiles @ attT
            with (
                tc.tile_pool(name="B_v", bufs=1) as pv,
                tc.tile_pool(name="B_at", bufs=1) as pat2,
                tc.tile_pool(name="B_st", bufs=4) as pst2,
                tc.tile_pool(name="B_ps2", bufs=4, space="PSUM") as pps2,
            ):
                vt = pv.tile([P, TO, D], F32R)
                nc.sync.dma_start(vt[:], v_d[:, :, h * D:(h + 1) * D])
                for tch in range(T // SC):
                    at = pat2.tile([P, TO, SC], F32R, tag="at")
                    nc.sync.dma_start(at[:], attT_d[:, :, tch * SC:(tch + 1) * SC])
                    for do in range(DO):
                        ps = pps2.tile([P, SC], F32, tag="psB2")
                        for so in range(TO):
                            nc.tensor.matmul(
                                ps[:], vt[:, so, do * P:(do + 1) * P], at[:, so],
                                start=(so == 0), stop=(so == TO - 1))
                        st = pst2.tile([P, SC], F32R, tag="stB2")
                        nc.vector.tensor_copy(st[:], ps[:])
                        nc.sync.dma_start(
                            aoT_d[:, h * DO + do, tch * SC:(tch + 1) * SC], st[:])

        # ---------------- Stage C: output projection ----------------
        for hf in range(2):
            t0 = hf * TH
            with (
                tc.tile_pool(name="C_a", bufs=1) as pa,
                tc.tile_pool(name="C_w", bufs=2) as pw,
                tc.tile_pool(name="C_st", bufs=4) as pst,
                tc.tile_pool(name="C_ps", bufs=4, space="PSUM") as pps,
            ):
                ah = pa.tile([P, CO, TH], F32R)
                nc.sync.dma_start(ah[:], aoT_d[:, :, t0:t0 + TH])
                for ech in range(C // EN):
                    wsl = pw.tile([P, CO, EN], F32R, tag="wo")
                    nc.sync.dma_start(wsl[:], woT[:, :, ech * EN:(ech + 1) * EN])
                    for tt in range(TH // P):
                        ps = pps.tile([P, EN], F32, tag="psC")
                        for fo in range(CO):
                            nc.tensor.matmul(
                                ps[:], ah[:, fo, tt * P:(tt + 1) * P], wsl[:, fo],
                                start=(fo == 0), stop=(fo == CO - 1))
                        st = pst.tile([P, EN], F32, tag="stC")
                        nc.vector.tensor_copy(st[:], ps[:])
                        nc.sync.dma_start(
                            y_o[t0 + tt * P: t0 + (tt + 1) * P, ech * EN:(ech + 1) * EN],
                            st[:])

    nc.compile()
    return nc


def build_noop_program(T=FULL_T, C=FULL_C, H=FULL_H, num_devices=8):
    """Same external IO as build_program but near-zero device work — used
    to estimate transfer/RPC overhead so it can be subtracted from wall
    time when reporting kernel execution time."""
    CO = C // P
    nc = bacc.Bacc("TRN2", num_devices=num_devices)
    nc.dram_tensor("xT", [P, CO, T], F32R, kind="ExternalInput")
    nc.dram_tensor("wqT", [P, CO, C], F32R, kind="ExternalInput")
    nc.dram_tensor("wkT", [P, CO, C], F32R, kind="ExternalInput")
    nc.dram_tensor("wvT", [P, CO, C], F32R, kind="ExternalInput")
    nc.dram_tensor("woT", [P, CO, C], F32R, kind="ExternalInput")
    bias_rep = nc.dram_tensor("bias_rep", [H, P, T], F32, kind="ExternalInput")
    atten_o = nc.dram_tensor("atten_o", [H, T, T], F32, kind="ExternalOutput")
    y_o = nc.dram_tensor("y_o", [T, C], F32, kind="ExternalOutput")
    with tile.TileContext(nc) as tc, ExitStack() as ctx:
        pool = ctx.enter_context(tc.tile_pool(name="sbuf", bufs=1))
        t = pool.tile([P, P], F32)
        nc.sync.dma_start(t[:], bias_rep[0, :, :P])
        nc.sync.dma_start(y_o[:P, :P], t[:])
        nc.sync.dma_start(atten_o[0, :P, :P], t[:])
    nc.compile()
    return nc


def prep_inputs(x, Wq, Wk, Wv, Wo, position_bias):
    """Host-side reshape/transpose into the device layouts. Returns the
    per-core input maps (weights shared by reference)."""
    x = np.asarray(x, dtype=np.float32)
    B, T, C = x.shape
    H = position_bias.shape[0]
    CO = C // P

    def wt_tiles(W):
        # [ci, co, d] with element = W[d, co*P+ci]
        return np.ascontiguousarray(
            np.asarray(W, dtype=np.float32).T.reshape(CO, P, C).transpose(1, 0, 2))

    wq_t = wt_tiles(Wq)
    wk_t = wt_tiles(Wk)
    wv_t = wt_tiles(Wv)
    wo_t = wt_tiles(Wo)
    bias_rep = np.ascontiguousarray(
        np.broadcast_to(np.asarray(position_bias, np.float32)[:, None, :], (H, P, T)))

    in_maps = []
    for b in range(B):
        xt = np.ascontiguousarray(
            x[b].T.reshape(CO, P, T).transpose(1, 0, 2))
        in_maps.append({
            "xT": xt, "wqT": wq_t, "wkT": wk_t, "wvT": wv_t, "woT": wo_t,
            "bias_rep": bias_rep,
        })
    return in_maps


_PROGRAM = None


def kernel(x, Wq, Wk, Wv, Wo, position_bias):
    global _PROGRAM
    x = np.asarray(x, dtype=np.float32)
    B, T, C = x.shape
    H = np.asarray(position_bias).shape[0]
    if _PROGRAM is None:
        _PROGRAM = build_program(T=T, C=C, H=H, num_devices=B)
    nc = _PROGRAM
    in_maps = prep_inputs(x, Wq, Wk, Wv, Wo, position_bias)
    res = run_bass_kernel_spmd(nc, in_maps, list(range(B)))
    y = np.stack([res.results[b]["y_o"] for b in range(B)])
    atten = np.stack([res.results[b]["atten_o"] for b in range(B)])
    return (y, atten)
